# revision 15
# baseline (speedup 1.0000x reference)
"""CSWin attention Bass/Trainium2 kernel (SPMD over 8 NeuronCores).

Problem: nn_CSWinAttention. B=2, H=W=56, N=2 candidates, C=128 channels,
8 heads x d=16, vertical-stripe windows Hsp=56, Wsp=7 -> 16 windows of
L=784 tokens. Plus LePE-style depthwise-3x3 rpe on the value.

Sharding: each core owns 2 windows (core c -> batch c//4, window cols
[14*(c%4), 14*(c%4)+14)). Full attention + rpe computed on-device; host
only slices/pads inputs and concatenates outputs.

Device algorithm per window (all fp32):
  - Q,K tiles [112,7,128] (token-chunk x ychunk x channel), PE-transposed
    into Q^T/K^T "even"/"odd" layouts so each head's 16 channels sit at a
    32-aligned partition base -> 2-way row-tiled (tile_position) QK^T
    matmuls with K-dim=16.
  - S^T chunks [112(q) x 784(p)] in PSUM, additive -1e30 pair-mask on the
    diagonal 112-block (DVE), exp via ACT (scale=1/4 folded in) -> P^T in
    SBUF.
  - AV: col-tiled (M=17) matmuls [V_h|1]^T @ P^T accumulate out^T + row
    sums in PSUM; rowsums broadcast to a divisor matrix via a one-hot
    matmul, fast-reciprocal, multiply.
  - rpe: PE-transpose V (with x halo) to [c, token], n-sum, 9-tap
    depthwise conv as scalar_tensor_tensor chain on GPSIMD, combine.
  - scaled attn + rpe transposed back per 112-chunk, summed, DMA'd out.
"""

import numpy as np

B, Hh, Ww, Nc, Cc = 2, 56, 56, 2, 128
HEADS, Dh, WSP = 8, 16, 7
L = Hh * WSP * Nc          # 784 tokens per window
PCH = 112                  # token chunk (partition) size; 7 chunks
QC = L // PCH              # 7
HALF0, HALF1 = 448, 336    # p split (4 chunks / 3 chunks), <=512 fp32 psum bank
SCALE = float(Dh) ** -0.5
NEGBIG = -1.0e30

_cache = {}


def _build_program():
    import concourse.bacc as bacc
    import concourse.tile as tile
    from concourse import mybir

    dt = mybir.dt.float32
    AT = mybir.AluOpType
    AF = mybir.ActivationFunctionType

    nc = bacc.Bacc("TRN2", target_bir_lowering=False, debug=False, num_devices=8)

    q_d = nc.dram_tensor("q", [Hh, 2 * WSP, Nc, Cc], dt, kind="ExternalInput")
    k_d = nc.dram_tensor("k", [Hh, 2 * WSP, Nc, Cc], dt, kind="ExternalInput")
    v_d = nc.dram_tensor("v", [Hh, 2 * WSP + 2, Nc, Cc], dt, kind="ExternalInput")
    tapw_d = nc.dram_tensor("tapw", [Cc, 9], dt, kind="ExternalInput")
    cneg_d = nc.dram_tensor("cneg", [Cc, 1], dt, kind="ExternalInput")
    cpos_d = nc.dram_tensor("cpos", [Cc, 1], dt, kind="ExternalInput")
    mask_d = nc.dram_tensor("maskblk", [PCH, PCH], dt, kind="ExternalInput")
    iden_d = nc.dram_tensor("iden", [Cc, Cc], dt, kind="ExternalInput")
    ebc_d = nc.dram_tensor("ebc", [Cc, Cc], dt, kind="ExternalInput")
    out_d = nc.dram_tensor("out", [Hh, 2 * WSP, Nc, Cc], dt, kind="ExternalOutput")

    with tile.TileContext(nc) as tc:
        with (
            tc.tile_pool(name="consts", bufs=1) as consts,
            tc.tile_pool(name="io", bufs=2) as io,
            tc.tile_pool(name="tr", bufs=2) as trp,
            tc.tile_pool(name="rpe", bufs=2) as rpep,
            tc.tile_pool(name="pt", bufs=8) as ptp,
            tc.tile_pool(name="post", bufs=2) as postp,
            tc.tile_pool(name="ps_st", bufs=4, space="PSUM") as ps_st,
            tc.tile_pool(name="ps_av", bufs=2, space="PSUM") as ps_av,
            tc.tile_pool(name="ps_misc", bufs=2, space="PSUM") as ps_misc,
        ):
            iden = consts.tile([Cc, Cc], dt)
            nc.sync.dma_start(out=iden[:], in_=iden_d[:])
            maskblk = consts.tile([PCH, PCH], dt)
            nc.sync.dma_start(out=maskblk[:], in_=mask_d[:])
            tapw = consts.tile([Cc, 9], dt)
            nc.sync.dma_start(out=tapw[:], in_=tapw_d[:])
            cneg = consts.tile([Cc, 1], dt)
            nc.sync.dma_start(out=cneg[:], in_=cneg_d[:])
            cpos = consts.tile([Cc, 1], dt)
            nc.sync.dma_start(out=cpos[:], in_=cpos_d[:])
            ebc = consts.tile([Cc, Cc], dt)
            nc.sync.dma_start(out=ebc[:], in_=ebc_d[:])

            for jj in range(2):  # two windows per core
                x0 = WSP * jj

                # ---------- loads ----------
                q_sb = io.tile([PCH, QC, Cc], dt, tag="q_sb")
                k_sb = io.tile([PCH, QC, Cc], dt, tag="k_sb")
                for c in range(QC):
                    nc.sync.dma_start(
                        out=q_sb[:, c, :],
                        in_=q_d[8 * c:8 * c + 8, x0:x0 + WSP, :, :].rearrange(
                            "y x n c -> y x (n c)"
                        ),
                    )
                    nc.sync.dma_start(
                        out=k_sb[:, c, :],
                        in_=k_d[8 * c:8 * c + 8, x0:x0 + WSP, :, :].rearrange(
                            "y x n c -> y x (n c)"
                        ),
                    )
                # V with per-head [16 cols | ones | pad] 24-blocks for AV lhsT
                v_aug = io.tile([PCH, QC, HEADS, 24], dt, tag="v_aug")
                for c in range(QC):
                    nc.sync.dma_start(
                        out=v_aug[:, c, :, 0:Dh],
                        in_=v_d[8 * c:8 * c + 8, 1 + x0:1 + x0 + WSP, :, :]
                        .rearrange("y x n (h d) -> y x (n h) d", h=HEADS),
                    )
                nc.vector.memset(v_aug[:, :, :, Dh:Dh + 1], 1.0)
                # V with x halo for the conv (126 = 7y * 9x * 2n)
                v_ext = io.tile([126, 8, Cc], dt, tag="v_ext")
                for b8 in range(8):
                    nc.sync.dma_start(
                        out=v_ext[:, b8, :],
                        in_=v_d[7 * b8:7 * b8 + 7, x0:x0 + WSP + 2, :, :]
                        .rearrange("y x n c -> y x (n c)"),
                    )

                # ---------- transposes ----------
                qt_ev = trp.tile([Cc, QC, PCH], dt, tag="qt_ev")
                qt_od = trp.tile([PCH, QC, PCH], dt, tag="qt_od")
                kt_ev = trp.tile([Cc, QC, PCH], dt, tag="kt_ev")
                kt_od = trp.tile([PCH, QC, PCH], dt, tag="kt_od")
                for c in range(QC):
                    for src, dst_ev, dst_od in ((q_sb, qt_ev, qt_od),
                                                (k_sb, kt_ev, kt_od)):
                        t1 = ps_misc.tile([Cc, PCH], dt, tag="tr")
                        nc.tensor.transpose(
                            t1[:], src[:, c, :], iden[0:PCH, 0:PCH]
                        )
                        nc.vector.tensor_copy(dst_ev[:, c, :], t1[:])
                        t2 = ps_misc.tile([PCH, PCH], dt, tag="tr")
                        nc.tensor.transpose(
                            t2[:], src[:, c, Dh:Cc], iden[0:PCH, 0:PCH]
                        )
                        nc.vector.tensor_copy(dst_od[:, c, :], t2[:])

                vt_ext = trp.tile([Cc, 8, 7, 9, 2], dt, tag="vt_ext")
                for b8 in range(8):
                    t3 = ps_misc.tile([Cc, 126], dt, tag="tr")
                    nc.tensor.transpose(
                        t3[:], v_ext[:, b8, :], iden[0:126, 0:126]
                    )
                    nc.vector.tensor_copy(
                        vt_ext[:, b8, :, :, :].rearrange("c y x n -> c (y x n)"),
                        t3[:],
                    )

                # ---------- rpe (GPSIMD) ----------
                vs_pad = rpep.tile([Cc, 58, 9], dt, tag="vs_pad")
                nc.gpsimd.memset(vs_pad[:], 0.0)
                nc.gpsimd.tensor_tensor(
                    vs_pad[:, 1:57, :].rearrange("c (yb y) x -> c yb y x", y=7),
                    vt_ext[:, :, :, :, 0],
                    vt_ext[:, :, :, :, 1],
                    AT.add,
                )
                conv_a = rpep.tile([Cc, 56, 7], dt, tag="conv_a")
                conv_b = rpep.tile([Cc, 56, 7], dt, tag="conv_b")
                acc_src = None
                for t in range(9):
                    ky, kx = t // 3, t % 3
                    shifted = vs_pad[:, ky:ky + 56, kx:kx + 7]
                    dst = conv_a if t % 2 == 0 else conv_b
                    if t == 0:
                        nc.vector.tensor_scalar(
                            dst[:], shifted, tapw[:, t:t + 1], None, AT.mult
                        )
                    else:
                        nc.vector.scalar_tensor_tensor(
                            dst[:], shifted, tapw[:, t:t + 1], acc_src[:],
                            AT.mult, AT.add,
                        )
                    acc_src = dst
                # cvs = conv - center*vs   (on interior x: vs_pad x 1..8)
                cvs = rpep.tile([Cc, 56, 7], dt, tag="cvs")
                nc.vector.scalar_tensor_tensor(
                    cvs[:], vs_pad[:, 1:57, 1:8], cneg[:], acc_src[:],
                    AT.mult, AT.add,
                )
                # rpe[c, y, x, n] = center*v + cvs
                rpe = rpep.tile([Cc, 56, 7, 2], dt, tag="rpe")
                for n in range(2):
                    nc.vector.scalar_tensor_tensor(
                        rpe[:, :, :, n],
                        vt_ext[:, :, :, 1:8, n].rearrange("c yb y x -> c (yb y) x"),
                        cpos[:],
                        cvs[:],
                        AT.mult, AT.add,
                    )

                # ---------- attention ----------
                # sets of 4 heads; each set: 2 row-tiled pairs
                scaled = {}
                for set_i in range(2):
                    heads = [4 * set_i + i for i in range(4)]
                    av0 = ps_av.tile([Cc, 512], dt, tag="av")
                    av1 = ps_av.tile([Cc, 512], dt, tag="av")
                    pairs = [
                        (kt_ev, qt_ev, (heads[0], heads[2])),
                        (kt_od, qt_od, (heads[1], heads[3])),
                    ]
                    def emit_av(qc, pt_by_head):
                        # 4 col-tiled matmuls back-to-back per half so the
                        # 32-col sub-arrays run them concurrently
                        for half, av, n in ((0, av0, HALF0), (1, av1, HALF1)):
                            for h in heads:
                                j = h % 4
                                pt = pt_by_head[h]
                                nc.tensor.matmul(
                                    av[32 * j:32 * j + Dh + 1, 0:n],
                                    v_aug[:, qc, h, 0:Dh + 1],
                                    pt[:, HALF0 * half:HALF0 * half + n],
                                    start=(qc == 0), stop=(qc == QC - 1),
                                    tile_position=(0, 32 * j),
                                    skip_group_check=True,
                                )

                    prev_pt = None
                    for qc in range(QC):
                        # AV for the previous chunk goes first: its inputs
                        # are long since ready, so the 4-head col-tiled
                        # quads issue back-to-back and overlap on the
                        # 32-col sub-arrays.
                        if prev_pt is not None:
                            emit_av(qc - 1, prev_pt)
                        pt_by_head = {}
                        for kt, qt, phs in pairs:
                            # row-group bases:
                            # set A ev (h0,h2)->(0,32); od (h1,h3)->(0,32)
                            # set B ev (h4,h6)->(64,96); od (h5,h7)->(64,96)
                            bases = (64 * set_i, 64 * set_i + 32)
                            sts = []
                            for h, base in zip(phs, bases):
                                st0 = ps_st.tile([PCH, 512], dt, tag="st")
                                st1 = ps_st.tile([PCH, 512], dt, tag="st")
                                sts.append((h, base, st0, st1))
                            # per-head serial matmuls; halves share one
                            # weight load, next head's load pulls ahead
                            # (different row group)
                            for h, base, st0, st1 in sts:
                                nc.tensor.matmul(
                                    st0[:, 0:HALF0],
                                    kt[base:base + Dh, qc, :],
                                    qt[base:base + Dh, 0:4, :]
                                    .rearrange("k a b -> k (a b)"),
                                    start=True, stop=True,
                                    tile_position=(base, 0),
                                )
                                nc.tensor.matmul(
                                    st1[:, 0:HALF1],
                                    kt[base:base + Dh, qc, :],
                                    qt[base:base + Dh, 4:7, :]
                                    .rearrange("k a b -> k (a b)"),
                                    start=True, stop=True,
                                    tile_position=(base, 0),
                                )
                            for h, base, st0, st1 in sts:
                                # pair mask on diagonal 112-block
                                if qc < 4:
                                    blk = st0[:, qc * PCH:(qc + 1) * PCH]
                                else:
                                    blk = st1[:, (qc - 4) * PCH:(qc - 3) * PCH]
                                nc.vector.tensor_tensor(
                                    blk, blk, maskblk[:], AT.add
                                )
                                pt = ptp.tile([PCH, L], dt, tag="pt")
                                nc.scalar.activation(
                                    pt[:, 0:HALF0], st0[:, 0:HALF0],
                                    AF.Exp, scale=SCALE,
                                )
                                nc.scalar.activation(
                                    pt[:, HALF0:L], st1[:, 0:HALF1],
                                    AF.Exp, scale=SCALE,
                                )
                                pt_by_head[h] = pt
                        prev_pt = pt_by_head
                    emit_av(QC - 1, prev_pt)

                    # ----- normalize: divisor broadcast + reciprocal -----
                    av_sb = postp.tile([Cc, L], dt, tag="av_sb")
                    nc.vector.tensor_copy(av_sb[:, 0:HALF0], av0[:, 0:HALF0])
                    nc.vector.tensor_copy(av_sb[:, HALF0:L], av1[:, 0:HALF1])
                    d0 = ps_av.tile([Cc, 512], dt, tag="av")
                    d1 = ps_av.tile([Cc, 512], dt, tag="av")
                    nc.tensor.matmul(d0[:, 0:HALF0], ebc[:], av_sb[:, 0:HALF0],
                                     start=True, stop=True)
                    nc.tensor.matmul(d1[:, 0:HALF1], ebc[:], av_sb[:, HALF0:L],
                                     start=True, stop=True)
                    drec = postp.tile([Cc, L], dt, tag="drec")
                    scr = postp.tile([Cc, L], dt, tag="scr")
                    nc.vector.reciprocal_approx_accurate(
                        drec[:, 0:HALF0], d0[:, 0:HALF0], scr[:, 0:HALF0]
                    )
                    nc.vector.reciprocal_approx_accurate(
                        drec[:, HALF0:L], d1[:, 0:HALF1], scr[:, HALF0:L]
                    )
                    sc = postp.tile([Cc, L], dt, tag="scaled")
                    nc.vector.tensor_tensor(sc[:], av_sb[:], drec[:], AT.mult)
                    scaled[set_i] = sc

                # ---------- final: transpose back, add rpe, store ----------
                final_sb = postp.tile([PCH, QC, Cc], dt, tag="final")
                rpe_flat = rpe[:].rearrange("c y x n -> c (y x n)")
                for qc in range(QC):
                    trr = ps_misc.tile([PCH, Cc], dt, tag="tr")
                    nc.tensor.transpose(
                        trr[:], rpe_flat[:, qc * PCH:(qc + 1) * PCH], iden[:]
                    )
                    rpe_tb = postp.tile([PCH, Cc], dt, tag="rpe_tb")
                    nc.vector.tensor_copy(rpe_tb[:], trr[:])
                    for set_i in range(2):
                        ta = ps_misc.tile([PCH, Cc], dt, tag="tr")
                        nc.tensor.transpose(
                            ta[:],
                            scaled[set_i][:, qc * PCH:(qc + 1) * PCH],
                            iden[:],
                        )
                        nc.vector.tensor_tensor(
                            final_sb[:, qc, :].rearrange(
                                "p (g k) -> p g k", g=HEADS
                            )[:, 4 * set_i:4 * set_i + 4, :],
                            ta[:].rearrange("p (g k) -> p g k", g=4)[:, :, 0:Dh],
                            rpe_tb[:].rearrange("p (g k) -> p g k", g=HEADS)[
                                :, 4 * set_i:4 * set_i + 4, :
                            ],
                            AT.add,
                        )
                for c in range(QC):
                    nc.sync.dma_start(
                        out=out_d[8 * c:8 * c + 8, x0:x0 + WSP, :, :].rearrange(
                            "y x n c -> y x (n c)"
                        ),
                        in_=final_sb[:, c, :],
                    )

    nc.compile()
    return nc


def _host_inputs(query, key, value, conv_w):
    """Build the 8 per-core input dicts."""
    query = np.ascontiguousarray(query, dtype=np.float32)
    key = np.ascontiguousarray(key, dtype=np.float32)
    value = np.ascontiguousarray(value, dtype=np.float32)
    conv_w = np.asarray(conv_w, dtype=np.float32)

    tapw = conv_w[:, 0].reshape(Cc, 9).copy()
    center = conv_w[:, 0, 1, 1].reshape(Cc, 1).copy()
    cneg = np.ascontiguousarray(-center)
    maskblk = np.zeros((PCH, PCH), np.float32)
    idx = np.arange(PCH)
    maskblk[idx, idx ^ 1] = NEGBIG
    iden = np.eye(Cc, dtype=np.float32)
    ebc = np.zeros((Cc, Cc), np.float32)
    for j in range(4):
        ebc[32 * j + Dh, 32 * j:32 * j + Dh] = 1.0

    in_maps = []
    for c in range(8):
        b, jblk = c // 4, c % 4
        xs = 14 * jblk
        v_sl = np.zeros((Hh, 16, Nc, Cc), np.float32)
        v_sl[:, 1:15] = value[b, :, xs:xs + 14]
        if xs - 1 >= 0:
            v_sl[:, 0] = value[b, :, xs - 1]
        if xs + 14 < Ww:
            v_sl[:, 15] = value[b, :, xs + 14]
        in_maps.append({
            "q": np.ascontiguousarray(query[b, :, xs:xs + 14]),
            "k": np.ascontiguousarray(key[b, :, xs:xs + 14]),
            "v": v_sl,
            "tapw": tapw,
            "cneg": cneg,
            "cpos": center,
            "maskblk": maskblk,
            "iden": iden,
            "ebc": ebc,
        })
    return in_maps


def _run(in_maps, trace=False):
    from concourse.bass_utils import run_bass_kernel_spmd

    if "nc" not in _cache:
        _cache["nc"] = _build_program()
    return run_bass_kernel_spmd(
        _cache["nc"], in_maps, core_ids=list(range(8)), trace=trace
    )


def kernel(query, key, value, conv_w):
    in_maps = _host_inputs(query, key, value, conv_w)
    res = _run(in_maps)
    out = np.zeros((B, Hh, Ww, Nc, Cc), np.float32)
    for c in range(8):
        b, jblk = c // 4, c % 4
        out[b, :, 14 * jblk:14 * jblk + 14] = res.results[c]["out"]
    return out


# revision 18
# speedup vs baseline: 1.0380x; 1.0380x over previous
"""CSWin attention Bass/Trainium2 kernel (SPMD over 8 NeuronCores).

Problem: nn_CSWinAttention. B=2, H=W=56, N=2 candidates, C=128 channels,
8 heads x d=16, vertical-stripe windows Hsp=56, Wsp=7 -> 16 windows of
L=784 tokens. Plus LePE-style depthwise-3x3 rpe on the value.

Sharding: each core owns 2 windows (core c -> batch c//4, window cols
[14*(c%4), 14*(c%4)+14)). Full attention + rpe computed on-device; host
only slices/pads inputs and concatenates outputs.

Device algorithm per window (all fp32):
  - Q,K tiles [112,7,128] (token-chunk x ychunk x channel), PE-transposed
    into Q^T/K^T "even"/"odd" layouts so each head's 16 channels sit at a
    32-aligned partition base -> 2-way row-tiled (tile_position) QK^T
    matmuls with K-dim=16.
  - S^T chunks [112(q) x 784(p)] in PSUM, additive -1e30 pair-mask on the
    diagonal 112-block (DVE), exp via ACT (scale=1/4 folded in) -> P^T in
    SBUF.
  - AV: col-tiled (M=17) matmuls [V_h|1]^T @ P^T accumulate out^T + row
    sums in PSUM; rowsums broadcast to a divisor matrix via a one-hot
    matmul, fast-reciprocal, multiply.
  - rpe: PE-transpose V (with x halo) to [c, token], n-sum, 9-tap
    depthwise conv as scalar_tensor_tensor chain on GPSIMD, combine.
  - scaled attn + rpe transposed back per 112-chunk, summed, DMA'd out.
"""

import numpy as np

B, Hh, Ww, Nc, Cc = 2, 56, 56, 2, 128
HEADS, Dh, WSP = 8, 16, 7
L = Hh * WSP * Nc          # 784 tokens per window
PCH = 112                  # token chunk (partition) size; 7 chunks
QC = L // PCH              # 7
HALF0, HALF1 = 448, 336    # p split (4 chunks / 3 chunks), <=512 fp32 psum bank
SCALE = float(Dh) ** -0.5
NEGBIG = -1.0e30

_cache = {}


def _build_program():
    import concourse.bacc as bacc
    import concourse.tile as tile
    from concourse import mybir

    dt = mybir.dt.float32
    AT = mybir.AluOpType
    AF = mybir.ActivationFunctionType

    nc = bacc.Bacc("TRN2", target_bir_lowering=False, debug=False, num_devices=8)

    q_d = nc.dram_tensor("q", [Hh, 2 * WSP, Nc, Cc], dt, kind="ExternalInput")
    k_d = nc.dram_tensor("k", [Hh, 2 * WSP, Nc, Cc], dt, kind="ExternalInput")
    v_d = nc.dram_tensor("v", [Hh, 2 * WSP + 2, Nc, Cc], dt, kind="ExternalInput")
    tapw_d = nc.dram_tensor("tapw", [Cc, 9], dt, kind="ExternalInput")
    cneg_d = nc.dram_tensor("cneg", [Cc, 1], dt, kind="ExternalInput")
    cpos_d = nc.dram_tensor("cpos", [Cc, 1], dt, kind="ExternalInput")
    mask_d = nc.dram_tensor("maskblk", [PCH, PCH], dt, kind="ExternalInput")
    iden_d = nc.dram_tensor("iden", [Cc, Cc], dt, kind="ExternalInput")
    ebc_d = nc.dram_tensor("ebc", [Cc, Cc], dt, kind="ExternalInput")
    out_d = nc.dram_tensor("out", [Hh, 2 * WSP, Nc, Cc], dt, kind="ExternalOutput")

    with tile.TileContext(nc) as tc:
        with (
            tc.tile_pool(name="consts", bufs=1) as consts,
            tc.tile_pool(name="io", bufs=2) as io,
            tc.tile_pool(name="tr", bufs=2) as trp,
            tc.tile_pool(name="rpe", bufs=2) as rpep,
            tc.tile_pool(name="pt", bufs=8) as ptp,
            tc.tile_pool(name="post", bufs=2) as postp,
            tc.tile_pool(name="ps_st", bufs=4, space="PSUM") as ps_st,
            tc.tile_pool(name="ps_av", bufs=2, space="PSUM") as ps_av,
            tc.tile_pool(name="ps_misc", bufs=2, space="PSUM") as ps_misc,
        ):
            iden = consts.tile([Cc, Cc], dt)
            nc.sync.dma_start(out=iden[:], in_=iden_d[:])
            maskblk = consts.tile([PCH, PCH], dt)
            nc.sync.dma_start(out=maskblk[:], in_=mask_d[:])
            tapw = consts.tile([Cc, 9], dt)
            nc.sync.dma_start(out=tapw[:], in_=tapw_d[:])
            cneg = consts.tile([Cc, 1], dt)
            nc.sync.dma_start(out=cneg[:], in_=cneg_d[:])
            cpos = consts.tile([Cc, 1], dt)
            nc.sync.dma_start(out=cpos[:], in_=cpos_d[:])
            ebc = consts.tile([Cc, Cc], dt)
            nc.sync.dma_start(out=ebc[:], in_=ebc_d[:])

            for jj in range(2):  # two windows per core
                x0 = WSP * jj

                # ---------- loads ----------
                q_sb = io.tile([PCH, QC, Cc], dt, tag="q_sb")
                k_sb = io.tile([PCH, QC, Cc], dt, tag="k_sb")
                for c in range(QC):
                    nc.sync.dma_start(
                        out=q_sb[:, c, :],
                        in_=q_d[8 * c:8 * c + 8, x0:x0 + WSP, :, :].rearrange(
                            "y x n c -> y x (n c)"
                        ),
                    )
                    nc.sync.dma_start(
                        out=k_sb[:, c, :],
                        in_=k_d[8 * c:8 * c + 8, x0:x0 + WSP, :, :].rearrange(
                            "y x n c -> y x (n c)"
                        ),
                    )
                # V with per-head [16 cols | ones | pad] 24-blocks for AV lhsT
                v_aug = io.tile([PCH, QC, HEADS, 24], dt, tag="v_aug")
                for c in range(QC):
                    nc.sync.dma_start(
                        out=v_aug[:, c, :, 0:Dh],
                        in_=v_d[8 * c:8 * c + 8, 1 + x0:1 + x0 + WSP, :, :]
                        .rearrange("y x n (h d) -> y x (n h) d", h=HEADS),
                    )
                nc.vector.memset(v_aug[:, :, :, Dh:Dh + 1], 1.0)
                # V with x halo for the conv (126 = 7y * 9x * 2n)
                v_ext = io.tile([126, 8, Cc], dt, tag="v_ext")
                for b8 in range(8):
                    nc.sync.dma_start(
                        out=v_ext[:, b8, :],
                        in_=v_d[7 * b8:7 * b8 + 7, x0:x0 + WSP + 2, :, :]
                        .rearrange("y x n c -> y x (n c)"),
                    )

                # ---------- transposes ----------
                qt_ev = trp.tile([Cc, QC, PCH], dt, tag="qt_ev")
                qt_od = trp.tile([PCH, QC, PCH], dt, tag="qt_od")
                kt_ev = trp.tile([Cc, QC, PCH], dt, tag="kt_ev")
                kt_od = trp.tile([PCH, QC, PCH], dt, tag="kt_od")
                for c in range(QC):
                    for src, dst_ev, dst_od in ((q_sb, qt_ev, qt_od),
                                                (k_sb, kt_ev, kt_od)):
                        t1 = ps_misc.tile([Cc, PCH], dt, tag="tr")
                        nc.tensor.transpose(
                            t1[:], src[:, c, :], iden[0:PCH, 0:PCH]
                        )
                        nc.vector.tensor_copy(dst_ev[:, c, :], t1[:])
                        t2 = ps_misc.tile([PCH, PCH], dt, tag="tr")
                        nc.tensor.transpose(
                            t2[:], src[:, c, Dh:Cc], iden[0:PCH, 0:PCH]
                        )
                        nc.vector.tensor_copy(dst_od[:, c, :], t2[:])

                vt_ext = trp.tile([Cc, 8, 7, 9, 2], dt, tag="vt_ext")
                for b8 in range(8):
                    t3 = ps_misc.tile([Cc, 126], dt, tag="tr")
                    nc.tensor.transpose(
                        t3[:], v_ext[:, b8, :], iden[0:126, 0:126]
                    )
                    nc.vector.tensor_copy(
                        vt_ext[:, b8, :, :, :].rearrange("c y x n -> c (y x n)"),
                        t3[:],
                    )

                # ---------- rpe (GPSIMD) ----------
                vs_pad = rpep.tile([Cc, 58, 9], dt, tag="vs_pad")
                nc.gpsimd.memset(vs_pad[:], 0.0)
                nc.gpsimd.tensor_tensor(
                    vs_pad[:, 1:57, :].rearrange("c (yb y) x -> c yb y x", y=7),
                    vt_ext[:, :, :, :, 0],
                    vt_ext[:, :, :, :, 1],
                    AT.add,
                )
                conv_a = rpep.tile([Cc, 56, 7], dt, tag="conv_a")
                conv_b = rpep.tile([Cc, 56, 7], dt, tag="conv_b")
                acc_src = None
                for t in range(9):
                    ky, kx = t // 3, t % 3
                    shifted = vs_pad[:, ky:ky + 56, kx:kx + 7]
                    dst = conv_a if t % 2 == 0 else conv_b
                    if t == 0:
                        nc.vector.tensor_scalar(
                            dst[:], shifted, tapw[:, t:t + 1], None, AT.mult
                        )
                    else:
                        nc.vector.scalar_tensor_tensor(
                            dst[:], shifted, tapw[:, t:t + 1], acc_src[:],
                            AT.mult, AT.add,
                        )
                    acc_src = dst
                # cvs = conv - center*vs   (on interior x: vs_pad x 1..8)
                cvs = rpep.tile([Cc, 56, 7], dt, tag="cvs")
                nc.vector.scalar_tensor_tensor(
                    cvs[:], vs_pad[:, 1:57, 1:8], cneg[:], acc_src[:],
                    AT.mult, AT.add,
                )
                # rpe[c, y, x, n] = center*v + cvs
                rpe = rpep.tile([Cc, 56, 7, 2], dt, tag="rpe")
                for n in range(2):
                    nc.vector.scalar_tensor_tensor(
                        rpe[:, :, :, n],
                        vt_ext[:, :, :, 1:8, n].rearrange("c yb y x -> c (yb y) x"),
                        cpos[:],
                        cvs[:],
                        AT.mult, AT.add,
                    )

                # ---------- attention ----------
                # sets of 4 heads; each set: 2 row-tiled pairs
                scaled = {}
                for set_i in range(2):
                    heads = [4 * set_i + i for i in range(4)]
                    av0 = ps_av.tile([Cc, 512], dt, tag="av")
                    av1 = ps_av.tile([Cc, 512], dt, tag="av")
                    pairs = [
                        (kt_ev, qt_ev, (heads[0], heads[2])),
                        (kt_od, qt_od, (heads[1], heads[3])),
                    ]
                    def emit_av(qc, pt_by_head):
                        # 4 col-tiled matmuls back-to-back per half so the
                        # 32-col sub-arrays run them concurrently
                        for half, av, n in ((0, av0, HALF0), (1, av1, HALF1)):
                            for h in heads:
                                j = h % 4
                                pt = pt_by_head[h]
                                nc.tensor.matmul(
                                    av[32 * j:32 * j + Dh + 1, 0:n],
                                    v_aug[:, qc, h, 0:Dh + 1],
                                    pt[:, HALF0 * half:HALF0 * half + n],
                                    start=(qc == 0), stop=(qc == QC - 1),
                                    tile_position=(0, 32 * j),
                                    skip_group_check=True,
                                )

                    prev_pt = None
                    for qc in range(QC):
                        # AV for the previous chunk goes first: its inputs
                        # are long since ready, so the 4-head col-tiled
                        # quads issue back-to-back and overlap on the
                        # 32-col sub-arrays.
                        if prev_pt is not None:
                            emit_av(qc - 1, prev_pt)
                        pt_by_head = {}
                        for kt, qt, phs in pairs:
                            # row-group bases:
                            # set A ev (h0,h2)->(0,32); od (h1,h3)->(0,32)
                            # set B ev (h4,h6)->(64,96); od (h5,h7)->(64,96)
                            bases = (64 * set_i, 64 * set_i + 32)
                            sts = []
                            for h, base in zip(phs, bases):
                                st0 = ps_st.tile([PCH, 512], dt, tag="st")
                                st1 = ps_st.tile([PCH, 512], dt, tag="st")
                                sts.append((h, base, st0, st1))
                            # per-head serial matmuls; halves share one
                            # weight load, next head's load pulls ahead
                            # (different row group)
                            for h, base, st0, st1 in sts:
                                nc.tensor.matmul(
                                    st0[:, 0:HALF0],
                                    kt[base:base + Dh, qc, :],
                                    qt[base:base + Dh, 0:4, :]
                                    .rearrange("k a b -> k (a b)"),
                                    start=True, stop=True,
                                    tile_position=(base, 0),
                                )
                                nc.tensor.matmul(
                                    st1[:, 0:HALF1],
                                    kt[base:base + Dh, qc, :],
                                    qt[base:base + Dh, 4:7, :]
                                    .rearrange("k a b -> k (a b)"),
                                    start=True, stop=True,
                                    tile_position=(base, 0),
                                )
                            for h, base, st0, st1 in sts:
                                pt = ptp.tile([PCH, L], dt, tag="pt")
                                nc.scalar.activation(
                                    pt[:, 0:HALF0], st0[:, 0:HALF0],
                                    AF.Exp, scale=SCALE,
                                )
                                nc.scalar.activation(
                                    pt[:, HALF0:L], st1[:, 0:HALF1],
                                    AF.Exp, scale=SCALE,
                                )
                                # pair mask: zero the partner entries in the
                                # diagonal 112-block, multiplicatively, off
                                # the PE<->ACT critical path (idle GPSIMD)
                                blk = pt[:, qc * PCH:(qc + 1) * PCH]
                                nc.gpsimd.tensor_tensor(
                                    blk, blk, maskblk[:], AT.mult
                                )
                                pt_by_head[h] = pt
                        prev_pt = pt_by_head
                    emit_av(QC - 1, prev_pt)

                    # ----- normalize: divisor broadcast + reciprocal -----
                    av_sb = postp.tile([Cc, L], dt, tag="av_sb")
                    nc.vector.tensor_copy(av_sb[:, 0:HALF0], av0[:, 0:HALF0])
                    nc.vector.tensor_copy(av_sb[:, HALF0:L], av1[:, 0:HALF1])
                    d0 = ps_av.tile([Cc, 512], dt, tag="av")
                    d1 = ps_av.tile([Cc, 512], dt, tag="av")
                    nc.tensor.matmul(d0[:, 0:HALF0], ebc[:], av_sb[:, 0:HALF0],
                                     start=True, stop=True)
                    nc.tensor.matmul(d1[:, 0:HALF1], ebc[:], av_sb[:, HALF0:L],
                                     start=True, stop=True)
                    drec = postp.tile([Cc, L], dt, tag="drec")
                    scr = postp.tile([Cc, L], dt, tag="scr")
                    nc.vector.reciprocal_approx_accurate(
                        drec[:, 0:HALF0], d0[:, 0:HALF0], scr[:, 0:HALF0]
                    )
                    nc.vector.reciprocal_approx_accurate(
                        drec[:, HALF0:L], d1[:, 0:HALF1], scr[:, HALF0:L]
                    )
                    sc = postp.tile([Cc, L], dt, tag="scaled")
                    nc.gpsimd.tensor_tensor(sc[:], av_sb[:], drec[:], AT.mult)
                    scaled[set_i] = sc

                # ---------- final: transpose back, add rpe, store ----------
                final_sb = postp.tile([PCH, QC, Cc], dt, tag="final")
                rpe_flat = rpe[:].rearrange("c y x n -> c (y x n)")
                for qc in range(QC):
                    trr = ps_misc.tile([PCH, Cc], dt, tag="tr")
                    nc.tensor.transpose(
                        trr[:], rpe_flat[:, qc * PCH:(qc + 1) * PCH], iden[:]
                    )
                    rpe_tb = postp.tile([PCH, Cc], dt, tag="rpe_tb")
                    nc.vector.tensor_copy(rpe_tb[:], trr[:])
                    for set_i in range(2):
                        ta = ps_misc.tile([PCH, Cc], dt, tag="tr")
                        nc.tensor.transpose(
                            ta[:],
                            scaled[set_i][:, qc * PCH:(qc + 1) * PCH],
                            iden[:],
                        )
                        nc.vector.tensor_tensor(
                            final_sb[:, qc, :].rearrange(
                                "p (g k) -> p g k", g=HEADS
                            )[:, 4 * set_i:4 * set_i + 4, :],
                            ta[:].rearrange("p (g k) -> p g k", g=4)[:, :, 0:Dh],
                            rpe_tb[:].rearrange("p (g k) -> p g k", g=HEADS)[
                                :, 4 * set_i:4 * set_i + 4, :
                            ],
                            AT.add,
                        )
                for c in range(QC):
                    nc.sync.dma_start(
                        out=out_d[8 * c:8 * c + 8, x0:x0 + WSP, :, :].rearrange(
                            "y x n c -> y x (n c)"
                        ),
                        in_=final_sb[:, c, :],
                    )

    nc.compile()
    return nc


def _host_inputs(query, key, value, conv_w):
    """Build the 8 per-core input dicts."""
    query = np.ascontiguousarray(query, dtype=np.float32)
    key = np.ascontiguousarray(key, dtype=np.float32)
    value = np.ascontiguousarray(value, dtype=np.float32)
    conv_w = np.asarray(conv_w, dtype=np.float32)

    tapw = conv_w[:, 0].reshape(Cc, 9).copy()
    center = conv_w[:, 0, 1, 1].reshape(Cc, 1).copy()
    cneg = np.ascontiguousarray(-center)
    maskblk = np.ones((PCH, PCH), np.float32)
    idx = np.arange(PCH)
    maskblk[idx, idx ^ 1] = 0.0
    iden = np.eye(Cc, dtype=np.float32)
    ebc = np.zeros((Cc, Cc), np.float32)
    for j in range(4):
        ebc[32 * j + Dh, 32 * j:32 * j + Dh] = 1.0

    in_maps = []
    for c in range(8):
        b, jblk = c // 4, c % 4
        xs = 14 * jblk
        v_sl = np.zeros((Hh, 16, Nc, Cc), np.float32)
        v_sl[:, 1:15] = value[b, :, xs:xs + 14]
        if xs - 1 >= 0:
            v_sl[:, 0] = value[b, :, xs - 1]
        if xs + 14 < Ww:
            v_sl[:, 15] = value[b, :, xs + 14]
        in_maps.append({
            "q": np.ascontiguousarray(query[b, :, xs:xs + 14]),
            "k": np.ascontiguousarray(key[b, :, xs:xs + 14]),
            "v": v_sl,
            "tapw": tapw,
            "cneg": cneg,
            "cpos": center,
            "maskblk": maskblk,
            "iden": iden,
            "ebc": ebc,
        })
    return in_maps


def _run(in_maps, trace=False):
    from concourse.bass_utils import run_bass_kernel_spmd

    if "nc" not in _cache:
        _cache["nc"] = _build_program()
    return run_bass_kernel_spmd(
        _cache["nc"], in_maps, core_ids=list(range(8)), trace=trace
    )


def kernel(query, key, value, conv_w):
    in_maps = _host_inputs(query, key, value, conv_w)
    res = _run(in_maps)
    out = np.zeros((B, Hh, Ww, Nc, Cc), np.float32)
    for c in range(8):
        b, jblk = c // 4, c % 4
        out[b, :, 14 * jblk:14 * jblk + 14] = res.results[c]["out"]
    return out


# revision 19
# speedup vs baseline: 1.0610x; 1.0221x over previous
"""CSWin attention Bass/Trainium2 kernel (SPMD over 8 NeuronCores).

Problem: nn_CSWinAttention. B=2, H=W=56, N=2 candidates, C=128 channels,
8 heads x d=16, vertical-stripe windows Hsp=56, Wsp=7 -> 16 windows of
L=784 tokens. Plus LePE-style depthwise-3x3 rpe on the value.

Sharding: each core owns 2 windows (core c -> batch c//4, window cols
[14*(c%4), 14*(c%4)+14)). Full attention + rpe computed on-device; host
only slices/pads inputs and concatenates outputs.

Device algorithm per window (all fp32):
  - Q,K tiles [112,7,128] (token-chunk x ychunk x channel), PE-transposed
    into Q^T/K^T "even"/"odd" layouts so each head's 16 channels sit at a
    32-aligned partition base -> 2-way row-tiled (tile_position) QK^T
    matmuls with K-dim=16.
  - S^T chunks [112(q) x 784(p)] in PSUM, additive -1e30 pair-mask on the
    diagonal 112-block (DVE), exp via ACT (scale=1/4 folded in) -> P^T in
    SBUF.
  - AV: col-tiled (M=17) matmuls [V_h|1]^T @ P^T accumulate out^T + row
    sums in PSUM; rowsums broadcast to a divisor matrix via a one-hot
    matmul, fast-reciprocal, multiply.
  - rpe: PE-transpose V (with x halo) to [c, token], n-sum, 9-tap
    depthwise conv as scalar_tensor_tensor chain on GPSIMD, combine.
  - scaled attn + rpe transposed back per 112-chunk, summed, DMA'd out.
"""

import numpy as np

B, Hh, Ww, Nc, Cc = 2, 56, 56, 2, 128
HEADS, Dh, WSP = 8, 16, 7
L = Hh * WSP * Nc          # 784 tokens per window
PCH = 112                  # token chunk (partition) size; 7 chunks
QC = L // PCH              # 7
HALF0, HALF1 = 448, 336    # p split (4 chunks / 3 chunks), <=512 fp32 psum bank
SCALE = float(Dh) ** -0.5
NEGBIG = -1.0e30

_cache = {}


def _build_program():
    import concourse.bacc as bacc
    import concourse.tile as tile
    from concourse import mybir

    dt = mybir.dt.float32
    AT = mybir.AluOpType
    AF = mybir.ActivationFunctionType

    nc = bacc.Bacc("TRN2", target_bir_lowering=False, debug=False, num_devices=8)

    q_d = nc.dram_tensor("q", [Hh, 2 * WSP, Nc, Cc], dt, kind="ExternalInput")
    k_d = nc.dram_tensor("k", [Hh, 2 * WSP, Nc, Cc], dt, kind="ExternalInput")
    v_d = nc.dram_tensor("v", [Hh, 2 * WSP + 2, Nc, Cc], dt, kind="ExternalInput")
    tapw_d = nc.dram_tensor("tapw", [Cc, 9], dt, kind="ExternalInput")
    cneg_d = nc.dram_tensor("cneg", [Cc, 1], dt, kind="ExternalInput")
    cpos_d = nc.dram_tensor("cpos", [Cc, 1], dt, kind="ExternalInput")
    mask_d = nc.dram_tensor("maskblk", [PCH, PCH], dt, kind="ExternalInput")
    iden_d = nc.dram_tensor("iden", [Cc, Cc], dt, kind="ExternalInput")
    ebc_d = nc.dram_tensor("ebc", [Cc, Cc], dt, kind="ExternalInput")
    out_d = nc.dram_tensor("out", [Hh, 2 * WSP, Nc, Cc], dt, kind="ExternalOutput")

    with tile.TileContext(nc) as tc:
        with (
            tc.tile_pool(name="consts", bufs=1) as consts,
            tc.tile_pool(name="io", bufs=2) as io,
            tc.tile_pool(name="tr", bufs=2) as trp,
            tc.tile_pool(name="rpe", bufs=2) as rpep,
            tc.tile_pool(name="pt", bufs=12) as ptp,
            tc.tile_pool(name="post", bufs=2) as postp,
            tc.tile_pool(name="ps_st", bufs=4, space="PSUM") as ps_st,
            tc.tile_pool(name="ps_av", bufs=2, space="PSUM") as ps_av,
            tc.tile_pool(name="ps_misc", bufs=2, space="PSUM") as ps_misc,
        ):
            iden = consts.tile([Cc, Cc], dt)
            nc.sync.dma_start(out=iden[:], in_=iden_d[:])
            maskblk = consts.tile([PCH, PCH], dt)
            nc.sync.dma_start(out=maskblk[:], in_=mask_d[:])
            tapw = consts.tile([Cc, 9], dt)
            nc.sync.dma_start(out=tapw[:], in_=tapw_d[:])
            cneg = consts.tile([Cc, 1], dt)
            nc.sync.dma_start(out=cneg[:], in_=cneg_d[:])
            cpos = consts.tile([Cc, 1], dt)
            nc.sync.dma_start(out=cpos[:], in_=cpos_d[:])
            ebc = consts.tile([Cc, Cc], dt)
            nc.sync.dma_start(out=ebc[:], in_=ebc_d[:])

            win = [dict() for _ in range(2)]
            for jj in range(2):  # loads + transposes + rpe, both windows
                x0 = WSP * jj

                # ---------- loads ----------
                q_sb = io.tile([PCH, QC, Cc], dt, tag="q_sb")
                k_sb = io.tile([PCH, QC, Cc], dt, tag="k_sb")
                for c in range(QC):
                    nc.sync.dma_start(
                        out=q_sb[:, c, :],
                        in_=q_d[8 * c:8 * c + 8, x0:x0 + WSP, :, :].rearrange(
                            "y x n c -> y x (n c)"
                        ),
                    )
                    nc.sync.dma_start(
                        out=k_sb[:, c, :],
                        in_=k_d[8 * c:8 * c + 8, x0:x0 + WSP, :, :].rearrange(
                            "y x n c -> y x (n c)"
                        ),
                    )
                # V with per-head [16 cols | ones | pad] 24-blocks for AV lhsT
                v_aug = io.tile([PCH, QC, HEADS, 24], dt, tag="v_aug")
                for c in range(QC):
                    nc.sync.dma_start(
                        out=v_aug[:, c, :, 0:Dh],
                        in_=v_d[8 * c:8 * c + 8, 1 + x0:1 + x0 + WSP, :, :]
                        .rearrange("y x n (h d) -> y x (n h) d", h=HEADS),
                    )
                nc.vector.memset(v_aug[:, :, :, Dh:Dh + 1], 1.0)
                # V with x halo for the conv (126 = 7y * 9x * 2n)
                v_ext = io.tile([126, 8, Cc], dt, tag="v_ext")
                for b8 in range(8):
                    nc.sync.dma_start(
                        out=v_ext[:, b8, :],
                        in_=v_d[7 * b8:7 * b8 + 7, x0:x0 + WSP + 2, :, :]
                        .rearrange("y x n c -> y x (n c)"),
                    )

                # ---------- transposes ----------
                qt_ev = trp.tile([Cc, QC, PCH], dt, tag="qt_ev")
                qt_od = trp.tile([PCH, QC, PCH], dt, tag="qt_od")
                kt_ev = trp.tile([Cc, QC, PCH], dt, tag="kt_ev")
                kt_od = trp.tile([PCH, QC, PCH], dt, tag="kt_od")
                for c in range(QC):
                    for src, dst_ev, dst_od in ((q_sb, qt_ev, qt_od),
                                                (k_sb, kt_ev, kt_od)):
                        t1 = ps_misc.tile([Cc, PCH], dt, tag="tr")
                        nc.tensor.transpose(
                            t1[:], src[:, c, :], iden[0:PCH, 0:PCH]
                        )
                        nc.vector.tensor_copy(dst_ev[:, c, :], t1[:])
                        t2 = ps_misc.tile([PCH, PCH], dt, tag="tr")
                        nc.tensor.transpose(
                            t2[:], src[:, c, Dh:Cc], iden[0:PCH, 0:PCH]
                        )
                        nc.vector.tensor_copy(dst_od[:, c, :], t2[:])

                vt_ext = trp.tile([Cc, 8, 7, 9, 2], dt, tag="vt_ext")
                for b8 in range(8):
                    t3 = ps_misc.tile([Cc, 126], dt, tag="tr")
                    nc.tensor.transpose(
                        t3[:], v_ext[:, b8, :], iden[0:126, 0:126]
                    )
                    nc.vector.tensor_copy(
                        vt_ext[:, b8, :, :, :].rearrange("c y x n -> c (y x n)"),
                        t3[:],
                    )

                # ---------- rpe (GPSIMD) ----------
                vs_pad = rpep.tile([Cc, 58, 9], dt, tag="vs_pad")
                nc.gpsimd.memset(vs_pad[:], 0.0)
                nc.gpsimd.tensor_tensor(
                    vs_pad[:, 1:57, :].rearrange("c (yb y) x -> c yb y x", y=7),
                    vt_ext[:, :, :, :, 0],
                    vt_ext[:, :, :, :, 1],
                    AT.add,
                )
                conv_a = rpep.tile([Cc, 56, 7], dt, tag="conv_a")
                conv_b = rpep.tile([Cc, 56, 7], dt, tag="conv_b")
                acc_src = None
                for t in range(9):
                    ky, kx = t // 3, t % 3
                    shifted = vs_pad[:, ky:ky + 56, kx:kx + 7]
                    dst = conv_a if t % 2 == 0 else conv_b
                    if t == 0:
                        nc.vector.tensor_scalar(
                            dst[:], shifted, tapw[:, t:t + 1], None, AT.mult
                        )
                    else:
                        nc.vector.scalar_tensor_tensor(
                            dst[:], shifted, tapw[:, t:t + 1], acc_src[:],
                            AT.mult, AT.add,
                        )
                    acc_src = dst
                # cvs = conv - center*vs   (on interior x: vs_pad x 1..8)
                cvs = rpep.tile([Cc, 56, 7], dt, tag="cvs")
                nc.vector.scalar_tensor_tensor(
                    cvs[:], vs_pad[:, 1:57, 1:8], cneg[:], acc_src[:],
                    AT.mult, AT.add,
                )
                # rpe[c, y, x, n] = center*v + cvs
                rpe = rpep.tile([Cc, 56, 7, 2], dt, tag="rpe")
                for n in range(2):
                    nc.vector.scalar_tensor_tensor(
                        rpe[:, :, :, n],
                        vt_ext[:, :, :, 1:8, n].rearrange("c yb y x -> c (yb y) x"),
                        cpos[:],
                        cvs[:],
                        AT.mult, AT.add,
                    )

                win[jj].update(q_sb=q_sb, k_sb=k_sb, v_aug=v_aug,
                               qt_ev=qt_ev, qt_od=qt_od, kt_ev=kt_ev,
                               kt_od=kt_od, rpe=rpe)

            for jj in range(2):  # attention + final, both windows
                x0 = WSP * jj
                q_sb = win[jj]["q_sb"]; k_sb = win[jj]["k_sb"]
                v_aug = win[jj]["v_aug"]
                qt_ev = win[jj]["qt_ev"]; qt_od = win[jj]["qt_od"]
                kt_ev = win[jj]["kt_ev"]; kt_od = win[jj]["kt_od"]
                rpe = win[jj]["rpe"]

                # ---------- attention ----------
                # sets of 4 heads; each set: 2 row-tiled pairs
                scaled = {}
                for set_i in range(2):
                    heads = [4 * set_i + i for i in range(4)]
                    av0 = ps_av.tile([Cc, 512], dt, tag="av")
                    av1 = ps_av.tile([Cc, 512], dt, tag="av")
                    pairs = [
                        (kt_ev, qt_ev, (heads[0], heads[2])),
                        (kt_od, qt_od, (heads[1], heads[3])),
                    ]
                    def emit_av(qc, pt_by_head):
                        # 4 col-tiled matmuls back-to-back per half so the
                        # 32-col sub-arrays run them concurrently
                        for half, av, n in ((0, av0, HALF0), (1, av1, HALF1)):
                            for h in heads:
                                j = h % 4
                                pt = pt_by_head[h]
                                nc.tensor.matmul(
                                    av[32 * j:32 * j + Dh + 1, 0:n],
                                    v_aug[:, qc, h, 0:Dh + 1],
                                    pt[:, HALF0 * half:HALF0 * half + n],
                                    start=(qc == 0), stop=(qc == QC - 1),
                                    tile_position=(0, 32 * j),
                                    skip_group_check=True,
                                )

                    prev_pt = None
                    for qc in range(QC):
                        # AV for the previous chunk goes first: its inputs
                        # are long since ready, so the 4-head col-tiled
                        # quads issue back-to-back and overlap on the
                        # 32-col sub-arrays.
                        if prev_pt is not None:
                            emit_av(qc - 1, prev_pt)
                        pt_by_head = {}
                        for kt, qt, phs in pairs:
                            # row-group bases:
                            # set A ev (h0,h2)->(0,32); od (h1,h3)->(0,32)
                            # set B ev (h4,h6)->(64,96); od (h5,h7)->(64,96)
                            bases = (64 * set_i, 64 * set_i + 32)
                            sts = []
                            for h, base in zip(phs, bases):
                                st0 = ps_st.tile([PCH, 512], dt, tag="st")
                                st1 = ps_st.tile([PCH, 512], dt, tag="st")
                                sts.append((h, base, st0, st1))
                            # per-head serial matmuls; halves share one
                            # weight load, next head's load pulls ahead
                            # (different row group)
                            for h, base, st0, st1 in sts:
                                nc.tensor.matmul(
                                    st0[:, 0:HALF0],
                                    kt[base:base + Dh, qc, :],
                                    qt[base:base + Dh, 0:4, :]
                                    .rearrange("k a b -> k (a b)"),
                                    start=True, stop=True,
                                    tile_position=(base, 0),
                                )
                                nc.tensor.matmul(
                                    st1[:, 0:HALF1],
                                    kt[base:base + Dh, qc, :],
                                    qt[base:base + Dh, 4:7, :]
                                    .rearrange("k a b -> k (a b)"),
                                    start=True, stop=True,
                                    tile_position=(base, 0),
                                )
                            for h, base, st0, st1 in sts:
                                pt = ptp.tile([PCH, L], dt, tag="pt")
                                nc.scalar.activation(
                                    pt[:, 0:HALF0], st0[:, 0:HALF0],
                                    AF.Exp, scale=SCALE,
                                )
                                nc.scalar.activation(
                                    pt[:, HALF0:L], st1[:, 0:HALF1],
                                    AF.Exp, scale=SCALE,
                                )
                                # pair mask: zero the partner entries in the
                                # diagonal 112-block, multiplicatively, off
                                # the PE<->ACT critical path (idle GPSIMD)
                                blk = pt[:, qc * PCH:(qc + 1) * PCH]
                                nc.gpsimd.tensor_tensor(
                                    blk, blk, maskblk[:], AT.mult
                                )
                                pt_by_head[h] = pt
                        prev_pt = pt_by_head
                    emit_av(QC - 1, prev_pt)

                    # ----- normalize: divisor broadcast + reciprocal -----
                    av_sb = postp.tile([Cc, L], dt, tag="av_sb")
                    nc.vector.tensor_copy(av_sb[:, 0:HALF0], av0[:, 0:HALF0])
                    nc.vector.tensor_copy(av_sb[:, HALF0:L], av1[:, 0:HALF1])
                    d0 = ps_av.tile([Cc, 512], dt, tag="av")
                    d1 = ps_av.tile([Cc, 512], dt, tag="av")
                    nc.tensor.matmul(d0[:, 0:HALF0], ebc[:], av_sb[:, 0:HALF0],
                                     start=True, stop=True)
                    nc.tensor.matmul(d1[:, 0:HALF1], ebc[:], av_sb[:, HALF0:L],
                                     start=True, stop=True)
                    drec = postp.tile([Cc, L], dt, tag="drec")
                    scr = postp.tile([Cc, L], dt, tag="scr")
                    nc.vector.reciprocal_approx_accurate(
                        drec[:, 0:HALF0], d0[:, 0:HALF0], scr[:, 0:HALF0]
                    )
                    nc.vector.reciprocal_approx_accurate(
                        drec[:, HALF0:L], d1[:, 0:HALF1], scr[:, HALF0:L]
                    )
                    sc = postp.tile([Cc, L], dt, tag="scaled")
                    nc.gpsimd.tensor_tensor(sc[:], av_sb[:], drec[:], AT.mult)
                    scaled[set_i] = sc

                # ---------- final: transpose back, add rpe, store ----------
                final_sb = postp.tile([PCH, QC, Cc], dt, tag="final")
                rpe_flat = rpe[:].rearrange("c y x n -> c (y x n)")
                for qc in range(QC):
                    trr = ps_misc.tile([PCH, Cc], dt, tag="tr")
                    nc.tensor.transpose(
                        trr[:], rpe_flat[:, qc * PCH:(qc + 1) * PCH], iden[:]
                    )
                    rpe_tb = postp.tile([PCH, Cc], dt, tag="rpe_tb")
                    nc.vector.tensor_copy(rpe_tb[:], trr[:])
                    for set_i in range(2):
                        ta = ps_misc.tile([PCH, Cc], dt, tag="tr")
                        nc.tensor.transpose(
                            ta[:],
                            scaled[set_i][:, qc * PCH:(qc + 1) * PCH],
                            iden[:],
                        )
                        nc.vector.tensor_tensor(
                            final_sb[:, qc, :].rearrange(
                                "p (g k) -> p g k", g=HEADS
                            )[:, 4 * set_i:4 * set_i + 4, :],
                            ta[:].rearrange("p (g k) -> p g k", g=4)[:, :, 0:Dh],
                            rpe_tb[:].rearrange("p (g k) -> p g k", g=HEADS)[
                                :, 4 * set_i:4 * set_i + 4, :
                            ],
                            AT.add,
                        )
                for c in range(QC):
                    nc.sync.dma_start(
                        out=out_d[8 * c:8 * c + 8, x0:x0 + WSP, :, :].rearrange(
                            "y x n c -> y x (n c)"
                        ),
                        in_=final_sb[:, c, :],
                    )

    nc.compile()
    return nc


def _host_inputs(query, key, value, conv_w):
    """Build the 8 per-core input dicts."""
    query = np.ascontiguousarray(query, dtype=np.float32)
    key = np.ascontiguousarray(key, dtype=np.float32)
    value = np.ascontiguousarray(value, dtype=np.float32)
    conv_w = np.asarray(conv_w, dtype=np.float32)

    tapw = conv_w[:, 0].reshape(Cc, 9).copy()
    center = conv_w[:, 0, 1, 1].reshape(Cc, 1).copy()
    cneg = np.ascontiguousarray(-center)
    maskblk = np.ones((PCH, PCH), np.float32)
    idx = np.arange(PCH)
    maskblk[idx, idx ^ 1] = 0.0
    iden = np.eye(Cc, dtype=np.float32)
    ebc = np.zeros((Cc, Cc), np.float32)
    for j in range(4):
        ebc[32 * j + Dh, 32 * j:32 * j + Dh] = 1.0

    in_maps = []
    for c in range(8):
        b, jblk = c // 4, c % 4
        xs = 14 * jblk
        v_sl = np.zeros((Hh, 16, Nc, Cc), np.float32)
        v_sl[:, 1:15] = value[b, :, xs:xs + 14]
        if xs - 1 >= 0:
            v_sl[:, 0] = value[b, :, xs - 1]
        if xs + 14 < Ww:
            v_sl[:, 15] = value[b, :, xs + 14]
        in_maps.append({
            "q": np.ascontiguousarray(query[b, :, xs:xs + 14]),
            "k": np.ascontiguousarray(key[b, :, xs:xs + 14]),
            "v": v_sl,
            "tapw": tapw,
            "cneg": cneg,
            "cpos": center,
            "maskblk": maskblk,
            "iden": iden,
            "ebc": ebc,
        })
    return in_maps


def _run(in_maps, trace=False):
    from concourse.bass_utils import run_bass_kernel_spmd

    if "nc" not in _cache:
        _cache["nc"] = _build_program()
    return run_bass_kernel_spmd(
        _cache["nc"], in_maps, core_ids=list(range(8)), trace=trace
    )


def kernel(query, key, value, conv_w):
    in_maps = _host_inputs(query, key, value, conv_w)
    res = _run(in_maps)
    out = np.zeros((B, Hh, Ww, Nc, Cc), np.float32)
    for c in range(8):
        b, jblk = c // 4, c % 4
        out[b, :, 14 * jblk:14 * jblk + 14] = res.results[c]["out"]
    return out
